# revision 1
# baseline (speedup 1.0000x reference)
"""Distributed SigLIP loss kernel for 8 trn2 NeuronCores.

loss*N = sum_ij softplus(L_ij) - sum_i L_ii,   L = exp(t')*(x_hat @ y_hat.T) + b

Sharding: img rows split 8 ways (2048 rows/core); every core holds full txt.
Per-core out-tiles are [128 txt-rows (partitions), 2048 img-cols (free)] so the
txt-row inv-norm rides the ACT per-partition scale and the img inv-norm (and
exp(t')) is pre-multiplied into the moving matmul operand.

softplus is composed as exp (ACT) -> pair-merge (DVE, w = ea*eb + ea + eb) ->
ln(w+1) (ACT, same exp/ln table set, accum_out per-partition reduction).
The host only reshapes/transposes/replicates inputs; all math runs on device.
"""

import sys
from contextlib import ExitStack

import numpy as np

try:
    import concourse.bass as bass  # noqa: F401
except ImportError:  # pragma: no cover
    sys.path.append("/opt/trn_rl_repo")
    import concourse.bass as bass  # noqa: F401

import concourse.mybir as mybir
import concourse.tile as tile
from concourse import bacc
from concourse.bass_utils import run_bass_kernel_spmd

# Keep Exp/Ln resolvable only via the combined natural_log_exp set, and Sqrt
# via a single set, so Bacc's table-load pass doesn't ping-pong table loads
# between exp-only and ln-only sets inside the main loop. Dict order (the
# act_func_set_id space) is preserved; only membership used for selection is
# narrowed, and the runtime tables genuinely contain the functions.
import functools as _functools

import concourse.hw_specs as _hw_specs


_ORIG_ACT_TABLES = _hw_specs.get_activation_tables.__wrapped__


@_functools.cache
def _patched_act_tables(module_arch):
    orig = _ORIG_ACT_TABLES(module_arch)
    _AF = mybir.ActivationFunctionType
    patched = {}
    for name, funcs in orig.items():
        funcs = set(funcs)
        if name != "natural_log_exp_and_others":
            funcs.discard(_AF.Exp)
            funcs.discard(_AF.Ln)
        if name != "sqrt_and_others":
            funcs.discard(_AF.Sqrt)
        patched[name] = funcs
    return patched


_hw_specs.get_activation_tables = _patched_act_tables
if getattr(bacc, "get_activation_tables", None) is not None:
    bacc.get_activation_tables = _patched_act_tables

N = 16384
D = 256
CORES = 8
SH = N // CORES          # 2048 img rows per core
NT = N // 128            # 128 txt-row tiles (out-tiles) per core
MT = SH // 128           # 16 row-tiles in the shard
LN_BATCH = 8             # out-tiles per ln instruction -> [128, 8192]
F32 = mybir.dt.float32
F16 = mybir.dt.float16
ADD = mybir.AluOpType.add
MULT = mybir.AluOpType.mult
SUB = mybir.AluOpType.subtract
AF = mybir.ActivationFunctionType

_CACHED_NC = None


def _build_nc():
    nc = bacc.Bacc(
        "TRN2",
        target_bir_lowering=False,
        debug=False,
        enable_asserts=False,
        num_devices=CORES,
    )
    txtT = nc.dram_tensor("txtT", [D, N], F16, kind="ExternalInput").ap()
    txtRF = nc.dram_tensor("txtRF", [N, D], F16, kind="ExternalInput").ap()
    txtRsh = nc.dram_tensor("txtRsh", [SH, D], F16, kind="ExternalInput").ap()
    imgT = nc.dram_tensor("imgT", [D, SH], F16, kind="ExternalInput").ap()
    imgR = nc.dram_tensor("imgR", [SH, D], F16, kind="ExternalInput").ap()
    tp = nc.dram_tensor("tp", [128, 1], F32, kind="ExternalInput").ap()
    bs = nc.dram_tensor("bs", [128, 1], F32, kind="ExternalInput").ap()
    out = nc.dram_tensor("out", [1, 1], F32, kind="ExternalOutput").ap()

    with tile.TileContext(nc) as tc, ExitStack() as ctx:
        big = ctx.enter_context(tc.tile_pool(name="big", bufs=1))
        rows = ctx.enter_context(tc.tile_pool(name="rows", bufs=32))
        scrp = ctx.enter_context(tc.tile_pool(name="scrp", bufs=4))
        ep = ctx.enter_context(tc.tile_pool(name="ep", bufs=3))
        uvp = ctx.enter_context(tc.tile_pool(name="uvp", bufs=3))
        lnp = ctx.enter_context(tc.tile_pool(name="lnp", bufs=2))
        small = ctx.enter_context(tc.tile_pool(name="small", bufs=1))
        psum = ctx.enter_context(tc.tile_pool(name="psum", bufs=2, space="PSUM"))

        # ---- resident loads --------------------------------------------
        # scalar (ACT) queue: small per-core inputs needed earliest
        tp_sb = small.tile([128, 1], F32, tag="tp")
        nc.scalar.dma_start(tp_sb[:], tp[:])
        bs_sb = small.tile([128, 1], F32, tag="bs")
        nc.scalar.dma_start(bs_sb[:], bs[:])
        imgT_sb = []
        for k in range(2):
            t = big.tile([128, SH], F16, tag=f"imgT{k}")
            nc.scalar.dma_start(t[:], imgT[128 * k : 128 * (k + 1), :])
            imgT_sb.append(t)
        # sync (SP) queue: imgR (needed first for scales), big txtT tiles,
        # then the norm-row stream
        imgR_sb = []
        for j in range(MT):
            r = big.tile([128, D], F16, tag=f"imgR{j}")
            nc.sync.dma_start(r[:], imgR[128 * j : 128 * (j + 1), :])
            imgR_sb.append(r)
        # txtT k0 on sync, k1 on scalar: the two 4MB loads ride different
        # HWDGE queues so tile-0's matmuls (which need both k-chunks) start
        # ~15us earlier
        txtT_sb = []
        for k, eng in ((0, nc.sync), (1, nc.scalar)):
            t = big.tile([128, N], F16, tag=f"txtT{k}")
            eng.dma_start(t[:], txtT[128 * k : 128 * (k + 1), :])
            txtT_sb.append(t)

        e_ap = small.tile([128, 1], F32, tag="eap")
        nc.scalar.activation(e_ap[:], tp_sb[:], AF.Exp)

        ones_col = small.tile([128, 1], F32, tag="onesc")
        nc.vector.memset(ones_col[:], 1.0)
        ones_row16 = small.tile([1, 128], F16, tag="onesr16")
        nc.vector.memset(ones_row16[:], 1.0)
        ident = small.tile([128, 128], F32, tag="ident")
        from concourse.masks import make_identity

        make_identity(nc, ident[:])

        # ---- img scales: norms from imgR rows (shared with the diagonal),
        # 1/sqrt(a) = exp(-0.5*ln(a)) keeps everything in the exp/ln table
        # set; the exp(t') factor folds into the Exp bias.
        nsqx = small.tile([128, MT], F32, tag="nsqx")
        for j in range(MT):
            s1 = scrp.tile([128, D], F32, tag="dscr")
            nc.vector.scalar_tensor_tensor(
                s1[:], imgR_sb[j][:], 1.0, imgR_sb[j][:], op0=MULT, op1=MULT,
                accum_out=nsqx[:, j : j + 1],
            )
        lx = small.tile([128, MT], F32, tag="lx")
        nc.scalar.activation(lx[:], nsqx[:], AF.Ln)
        sxm = small.tile([128, MT], F32, tag="sxm")
        nc.scalar.activation(
            sxm[:], lx[:], AF.Exp, bias=tp_sb[:], scale=-0.5
        )
        ix = small.tile([128, MT], F32, tag="ix")
        nc.scalar.activation(ix[:], lx[:], AF.Exp, scale=-0.5)
        # transpose to free layout and broadcast across partitions via PE
        sxm_ps = psum.tile([16, 128], F32, tag="mm")
        nc.tensor.transpose(sxm_ps[:], sxm[:], ident[:])
        s16 = small.tile([16, 128], F16, tag="s16")
        nc.vector.tensor_copy(s16[:], sxm_ps[:])
        s1row = small.tile([1, SH], F16, tag="s1row")
        nc.gpsimd.dma_start(s1row[:], s16[:])
        sb_ps = psum.tile([128, SH], F32, tag="mm")
        for g in range(SH // 512):
            nc.tensor.matmul(
                sb_ps[:, 512 * g : 512 * (g + 1)],
                lhsT=ones_row16[:],
                rhs=s1row[:, 512 * g : 512 * (g + 1)],
                start=True,
                stop=True,
            )
        imgTs = []
        for k in range(2):
            t = big.tile([128, SH], F16, tag=f"imgTs{k}")
            nc.vector.tensor_tensor(t[:], imgT_sb[k][:], sb_ps[:], op=MULT)
            imgTs.append(t)

        # ---- main loop, with txt-norm chunks interleaved ----------------
        # txt norms stream on the sync queue behind the txtT loads; inv-norm
        # chunks are computed right before the 16 exps that consume them so
        # the ACT FIFO never blocks on far-future norms.
        NCHUNK = 16
        nsq = small.tile([128, NT], F32, tag="nsq")
        rinv_n = small.tile([128, NT], F32, tag="rinvn")
        invn = small.tile([128, NT], F32, tag="invn")
        acc = small.tile([128, NT // LN_BATCH], F32, tag="acc")
        half = SH // 2
        quart = SH // 4
        eighth = SH // 8
        lnt = None
        pending_ln = None

        def _norm_stt(u):
            rt = rows.tile([128, D], F16, tag="trow")
            nc.sync.dma_start(rt[:], txtRF[128 * u : 128 * (u + 1), :])
            scr = scrp.tile([128, D], F32, tag="ttrs")
            nc.vector.scalar_tensor_tensor(
                scr[:], rt[:], 1.0, rt[:], op0=MULT, op1=MULT,
                accum_out=nsq[:, u : u + 1],
            )

        def _finish_norm_chunk(c):
            cs = slice(NCHUNK * c, NCHUNK * (c + 1))
            nc.scalar.activation(rinv_n[:, cs], nsq[:, cs], AF.Ln)
            nc.scalar.activation(invn[:, cs], rinv_n[:, cs], AF.Exp, scale=-0.5)

        for u in range(NCHUNK):
            _norm_stt(u)
        _finish_norm_chunk(0)
        for t in range(NT):
            # stream the next chunk's norm reductions two per tile across
            # tiles 4..11 of the current chunk, finish at tile 12 so the
            # chunk's Ln/Exp never sits at the ACT FIFO head
            h = t % NCHUNK - 4
            c_next = t // NCHUNK + 1
            if 0 <= h < 8 and c_next < NT // NCHUNK:
                u0 = NCHUNK * c_next + 2 * h
                _norm_stt(u0)
                _norm_stt(u0 + 1)
            if t % NCHUNK == 12 and c_next < NT // NCHUNK:
                _finish_norm_chunk(c_next)
            ps = psum.tile([128, SH], F32, tag="mm")
            for k in range(2):
                for j in range(SH // 512):
                    sl = slice(512 * j, 512 * (j + 1))
                    nc.tensor.matmul(
                        ps[:, sl],
                        lhsT=txtT_sb[k][:, 128 * t : 128 * (t + 1)],
                        rhs=imgTs[k][:, sl],
                        start=(k == 0),
                        stop=(k == 1),
                    )
            et = ep.tile([128, SH], F16, tag="et")
            nc.scalar.activation(
                et[:], ps[:], AF.Exp, bias=bs_sb[:], scale=invn[:, t : t + 1]
            )
            if pending_ln is not None and t % LN_BATCH == 2:
                pending_ln()
                pending_ln = None
            if t % LN_BATCH == 0:
                lnt = lnp.tile([128, eighth * LN_BATCH], F16, tag="lnt")
            # p = (1+e^l)/4 (two-op tensor_scalar, 4x), then three
            # pair-product levels (tensor_tensor, 2x): ln of the 8-way
            # product sums 8 softplus terms minus the constant 16*ln2
            # (corrected exactly in the final reduction); the /4 keeps all
            # products within fp16 range.
            p = uvp.tile([128, SH], F16, tag="p")
            nc.vector.tensor_scalar(p[:], et[:], 1.0, 0.25, op0=ADD, op1=MULT)
            m1 = uvp.tile([128, half], F16, tag="m1")
            nc.vector.tensor_tensor(m1[:], p[:, 0:half], p[:, half:SH], op=MULT)
            m2 = uvp.tile([128, quart], F16, tag="m2")
            nc.vector.tensor_tensor(m2[:], m1[:, 0:quart], m1[:, quart:half], op=MULT)
            msl = lnt[:, eighth * (t % LN_BATCH) : eighth * (t % LN_BATCH + 1)]
            nc.vector.tensor_tensor(msl, m2[:, 0:eighth], m2[:, eighth:quart], op=MULT)
            if t % LN_BATCH == LN_BATCH - 1:
                b_idx = t // LN_BATCH
                cur_lnt = lnt

                def _emit_ln(cur_lnt=cur_lnt, b_idx=b_idx):
                    nc.scalar.activation(
                        cur_lnt[:],
                        cur_lnt[:],
                        AF.Ln,
                        accum_out=acc[:, b_idx : b_idx + 1],
                    )

                pending_ln = _emit_ln
        if pending_ln is not None:
            pending_ln()
            pending_ln = None

        # ---- shard diagonal terms (tail; overlaps main-loop drain) -----
        nsqy = small.tile([128, MT], F32, tag="nsqy")
        dots = small.tile([128, MT], F32, tag="dots")
        for j in range(MT):
            tr = scrp.tile([128, D], F16, tag="srow_t")
            nc.sync.dma_start(tr[:], txtRsh[128 * j : 128 * (j + 1), :])
            s2 = scrp.tile([128, D], F32, tag="dscr")
            nc.vector.scalar_tensor_tensor(
                s2[:], tr[:], 1.0, tr[:], op0=MULT, op1=MULT,
                accum_out=nsqy[:, j : j + 1],
            )
            s3 = scrp.tile([128, D], F32, tag="dscr")
            nc.vector.scalar_tensor_tensor(
                s3[:], imgR_sb[j][:], 1.0, tr[:], op0=MULT, op1=MULT,
                accum_out=dots[:, j : j + 1],
            )
        ry = small.tile([128, MT], F32, tag="ry")
        nc.scalar.activation(ry[:], nsqy[:], AF.Ln)
        iy = small.tile([128, MT], F32, tag="iy")
        nc.scalar.activation(iy[:], ry[:], AF.Exp, scale=-0.5)
        sim = small.tile([128, MT], F32, tag="sim")
        nc.vector.tensor_tensor(sim[:], dots[:], ix[:], op=MULT)
        sim2 = small.tile([128, MT], F32, tag="sim2")
        nc.vector.tensor_tensor(sim2[:], sim[:], iy[:], op=MULT)
        dsum = small.tile([128, 1], F32, tag="dsum")
        nc.vector.reduce_sum(dsum[:], sim2[:], axis=mybir.AxisListType.X)

        # ---- final reduction -------------------------------------------
        A = small.tile([128, 1], F32, tag="A")
        nc.vector.reduce_sum(A[:], acc[:], axis=mybir.AxisListType.X)
        t1 = small.tile([128, 1], F32, tag="t1")
        nc.vector.tensor_tensor(t1[:], dsum[:], e_ap[:], op=MULT)
        t2 = small.tile([128, 1], F32, tag="t2")
        nc.vector.tensor_scalar(t2[:], bs_sb[:], float(MT), None, op0=MULT)
        t3 = small.tile([128, 1], F32, tag="t3")
        nc.vector.tensor_tensor(t3[:], t1[:], t2[:], op=ADD)
        corr = float((SH * N / 128 / 8) * 16.0 * np.log(2.0))
        A2 = small.tile([128, 1], F32, tag="A2")
        nc.vector.tensor_scalar(A2[:], A[:], corr, None, op0=ADD)
        C = small.tile([128, 1], F32, tag="C")
        nc.vector.tensor_tensor(C[:], A2[:], t3[:], op=SUB)
        fin_ps = psum.tile([1, 1], F32, tag="mm")
        nc.tensor.matmul(
            fin_ps[:], lhsT=ones_col[:], rhs=C[:], start=True, stop=True
        )
        fin = small.tile([1, 1], F32, tag="fin")
        nc.vector.tensor_copy(fin[:], fin_ps[:])
        nc.sync.dma_start(out[:], fin[:])

    nc.compile()
    return nc


def _get_nc():
    global _CACHED_NC
    if _CACHED_NC is None:
        _CACHED_NC = _build_nc()
    return _CACHED_NC


def _make_in_maps(img, txt, t_prime, bias):
    img = np.asarray(img, dtype=np.float32)
    txt = np.asarray(txt, dtype=np.float32)
    tpv = float(np.asarray(t_prime, dtype=np.float32))
    bsv = float(np.asarray(bias, dtype=np.float32))

    txt16 = txt.astype(np.float16)
    txtT = np.ascontiguousarray(txt16.T)            # [D, N]
    img16 = img.astype(np.float16)
    imgT = np.ascontiguousarray(img16.T)            # [D, N]

    tp_arr = np.full((128, 1), tpv, dtype=np.float32)
    bs_arr = np.full((128, 1), bsv, dtype=np.float32)

    in_maps = []
    for c in range(CORES):
        sl = slice(SH * c, SH * (c + 1))
        in_maps.append(
            {
                "txtT": txtT,
                "txtRF": txt16,
                "txtRsh": np.ascontiguousarray(txt16[sl]),
                "imgT": np.ascontiguousarray(imgT[:, sl]),
                "imgR": np.ascontiguousarray(img16[sl]),
                "tp": tp_arr,
                "bs": bs_arr,
            }
        )
    return in_maps


def _run(img, txt, t_prime, bias, trace=False):
    nc = _get_nc()
    in_maps = _make_in_maps(img, txt, t_prime, bias)
    res = run_bass_kernel_spmd(
        nc, in_maps, core_ids=list(range(CORES)), trace=trace
    )
    partials = [float(r["out"][0, 0]) for r in res.results]
    loss = np.float32(sum(partials) / N)
    return loss, res


def kernel(img, txt, t_prime, bias):
    loss, _ = _run(img, txt, t_prime, bias, trace=False)
    return np.asarray(loss, dtype=np.float32)

